# revision 38
# baseline (speedup 1.0000x reference)
"""2-layer GCN (GCNConv -> ReLU -> GCNConv) on 8 TRN2 NeuronCores.

Strategy (sliced-ELLPACK with self-slots, node sharding, bf16, wide DVE ops):
  GCN algebra: out = S relu(S x W1 + b1) W2 + b2 with S = D^-1/2 (A+I) D^-1/2.
  Normalization is separable (norm_e = dinv[row]*dinv[col]) and aggregation
  linear, so each layer is: per-slot scale -> unweighted neighbor-sum ->
  per-node scale -> tiny dense MLP. Self-loops are ordinary slots, so there
  is no separate self-term path.

  Host does pure index layout (no model math): sort nodes by degree, pack
  each node's incoming edges (+1 self slot) into padded slot rows (sliced
  ELLPACK, slices of 128 nodes, DP-chosen width groups). Slot values are the
  *input* features x[row] (bf16) and integer deg[row] (uint8, exact), packed
  into one byte table per DMA chunk (4B-aligned sections, large
  per-partition descriptors); chunk 1 also carries the per-node degs so a
  single rsqrt activation yields both the slot scales and dinv. Device does
  all FP math: rsqrt on the scalar engine (one act-table load, patched so
  Relu shares the abs_reciprocal_sqrt table set), slot scaling (broadcast
  mult) + pair-add + half-width segmented reduction on DVE, then the
  2->16->1 MLP as 16 fused stt ops + relus pipelined on the scalar engine +
  16 fused accumulate ops (DVE-saturated; scalar rides along).

  Two launches: A computes layer 1 + q = dinv * (h @ W2) and exports dinv;
  host re-shards q into the same slot layout (index gather only); B reduces
  q-slots and finishes layer 2 with two DVE ops (no scalar engine at all).
"""
import numpy as np
import ml_dtypes

BF16 = ml_dtypes.bfloat16
P = 128
N_CORES = 8
NSL = 246            # local slices per core (246*8*128 = 251904 >= 250k)
NRANKS = 2048 * P    # padded rank space (262144 >= 250000)
LAM = 60             # DP penalty (columns) per extra width-group
FRACS_A = (0.10, 0.33, 0.63, 1.01)   # graded DMA chunks, program A
FRACS_B = (0.20, 0.55, 1.01)   # program B

TRACE = False
_cache = {}


def _install_ntff_shim():
    import contextlib, ctypes, sys, types
    if "antenv.axon_hooks" in sys.modules:
        return
    try:
        lib = ctypes.CDLL("/opt/axon/libaxon_pjrt.so")
        if not hasattr(lib, "axon_start_nrt_profile"):
            return
        lib.axon_start_nrt_profile.argtypes = [ctypes.POINTER(ctypes.c_int64), ctypes.c_size_t]
        lib.axon_start_nrt_profile.restype = ctypes.c_int64
        lib.axon_stop_nrt_profile.argtypes = [ctypes.c_char_p]
        lib.axon_stop_nrt_profile.restype = ctypes.c_int64
    except OSError:
        return

    @contextlib.contextmanager
    def _hook(output_dir, device_ids):
        import jax
        jax.devices()
        if device_ids:
            ids = (ctypes.c_int64 * len(device_ids))(*device_ids)
            rc = lib.axon_start_nrt_profile(ids, len(device_ids))
        else:
            rc = lib.axon_start_nrt_profile(None, 0)
        if rc != 0:
            raise RuntimeError(f"axon_start_nrt_profile rc={rc}")
        try:
            yield
        finally:
            lib.axon_stop_nrt_profile(str(output_dir).encode())

    mod = types.ModuleType("antenv.axon_hooks")
    mod.get_axon_ntff_profile_hook = lambda: _hook
    mod.set_axon_ntff_profile_hook = lambda h: None
    sys.modules["antenv.axon_hooks"] = mod


def _plan_groups(W_l):
    """DP partition of local slices into contiguous width-groups.
    Boundaries restricted to even slice indices so every group block has an
    even column count (keeps bf16 sub-blocks 4B-aligned for DVE 2x mode)."""
    nsl = len(W_l)
    INF = 1 << 60
    best = np.full(nsl + 1, INF, np.int64)
    best[0] = 0
    ch = np.zeros(nsl + 1, np.int64)
    for e in range(2, nsl + 1, 2):
        s = np.arange(0, e, 2)
        c = best[s] + (e - s) * W_l[s] + LAM
        i = int(np.argmin(c))
        best[e] = c[i]
        ch[e] = 2 * i
    bnds = []
    e = nsl
    while e > 0:
        s = int(ch[e])
        bnds.append((s, e))
        e = s
    bnds.reverse()
    groups = []
    off = 0
    for s, e in bnds:
        w = int(W_l[s])
        groups.append((s, e, w, off))   # (l0, l1, width, column offset)
        off += (e - s) * w
    return groups, off                  # TOT = off


def _chunk_groups(groups, fracs):
    """Split groups into DMA chunks with graded sizes (small first chunk so
    compute starts as early as possible while later transfers overlap)."""
    tot = sum((e - s) * w for s, e, w, _ in groups)
    chunks = []
    cur = []
    acc = 0
    for g in groups:
        s, e, w, _ = g
        cur.append(g)
        acc += (e - s) * w
        if len(chunks) < len(fracs) - 1 and acc >= fracs[len(chunks)] * tot:
            chunks.append(cur)
            cur = []
    if cur:
        chunks.append(cur)
    return [c for c in chunks if c]


def _patch_act_tables():
    """Route both Relu and Abs_reciprocal_sqrt to the single act-func set
    that contains them both, so only one ACT_TABLE_LOAD (1.28us) is emitted.
    Entries before the target set are hidden (emptied) so positional
    act_func_set_ids stay aligned with act_info.json."""
    import concourse.bacc as bacc
    from concourse.hw_specs import get_activation_tables as _gat
    if getattr(bacc, "_act_tables_patched", False):
        return
    def patched(arch):
        tabs = _gat(arch)
        out = {}
        seen = False
        for k, v in tabs.items():
            if k == "abs_reciprocal_sqrt_and_small":
                out[k] = v
                seen = True
            elif not seen:
                out[k] = set()
            else:
                out[k] = v
        return out
    bacc.get_activation_tables = patched
    bacc._act_tables_patched = True


def _build_programs(groups, TOT):
    import concourse.bass as bass
    import concourse.bacc as bacc
    import concourse.tile as tile
    import concourse.mybir as mybir

    _patch_act_tables()

    f32 = mybir.dt.float32
    bf16 = mybir.dt.bfloat16
    AF = mybir.ActivationFunctionType
    ALU = mybir.AluOpType
    X = mybir.AxisListType.X
    chunksA = _chunk_groups(groups, FRACS_A)
    chunksB = _chunk_groups(groups, FRACS_B)

    # ---------------- program A ----------------
    ncA = bacc.Bacc("TRN2", target_bir_lowering=False, debug=False, num_devices=N_CORES)
    u8 = mybir.dt.uint8
    # packed per-chunk byte table: [deg u8 (ccols) | x0x1 bf16 (4*ccols B)]
    abt = ncA.dram_tensor("abt", [P, 5 * TOT + NSL + 16], u8, kind="ExternalInput")
    wb = ncA.dram_tensor("wb", [P, 65], f32, kind="ExternalInput")
    qd = ncA.dram_tensor("qd", [P, 2 * NSL], bf16, kind="ExternalOutput")  # q | dinv

    with tile.TileContext(ncA) as tc:
        with tc.tile_pool(name="slots", bufs=4) as pool, \
             tc.tile_pool(name="persist", bufs=1) as pp:
            # one packed transfer per chunk (large per-partition descriptors);
            # x section padded to a 4-byte boundary for DVE fast modes
            cts = []
            boff = 0
            for ci, cgroups in enumerate(chunksA):
                ccols = sum((e - s) * w for s, e, w, _ in cgroups)
                extra = NSL if ci == 0 else 0    # chunk 1 carries node degs
                aoff = (ccols + extra + 3) & ~3
                stride = aoff + 4 * ccols
                ct = pool.tile([P, stride], u8, tag="ab")
                if ci == 0:
                    # deg+nodedeg section first so rt1/dinv start early,
                    # x section follows on the same queue
                    ncA.sync.dma_start(ct[:, 0:aoff],
                                       abt.ap()[:, boff:boff + aoff])
                    ncA.sync.dma_start(ct[:, aoff:stride],
                                       abt.ap()[:, boff + aoff:boff + stride])
                else:
                    ncA.sync.dma_start(ct[:], abt.ap()[:, boff:boff + stride])
                cts.append(ct)
                boff += stride
            wbt = pp.tile([P, 65], f32)
            ncA.scalar.dma_start(wbt[:], wb.ap())
            agg = pp.tile([P, 2 * NSL], bf16)
            ncA.gpsimd.memset(agg[:], 0.0)

            # per-j MLP constants: relu scale ta_j = a_j,
            # ratio_j = b_j / a_j (u_j = z0 + ratio_j z1 folded via relu scale)
            ta = pp.tile([P, 16], f32)
            ncA.vector.tensor_scalar(out=ta[:], in0=wbt[:, 0:16], scalar1=1e-30,
                                     scalar2=None, op0=ALU.add)
            ratio = pp.tile([P, 16], f32)
            ncA.vector.reciprocal(ratio[:], ta[:])
            ncA.vector.tensor_tensor(out=ratio[:], in0=wbt[:, 16:32], in1=ratio[:],
                                     op=ALU.mult)

            # slot chunks: rsqrt(deg[row]) on scalar, scale + segmented-sum
            # on DVE. x01s chunk layout: [x0 of chunk | x1 of chunk]
            dinv = None
            for ci, cgroups in enumerate(chunksA):
                c0 = cgroups[0][3]
                ccols = sum((e - s) * w for s, e, w, _ in cgroups)
                ct = cts[ci]
                extra = NSL if ci == 0 else 0
                aoff = (ccols + extra + 3) & ~3
                dt = ct[:, 0:ccols + extra]
                xt = ct[:, aoff:aoff + 4 * ccols].bitcast(bf16)
                rt = pool.tile([P, ccols + extra], bf16, tag="r")
                if ci == 0:
                    # slot scales first (gates scale1); node dinv separately
                    # (needed only at z time, exported early off-queue)
                    ncA.scalar.activation(rt[:, 0:ccols], ct[:, 0:ccols],
                                          AF.Abs_reciprocal_sqrt)
                    dinv = rt[:, ccols:ccols + NSL]
                    ncA.scalar.activation(dinv, ct[:, ccols:ccols + NSL],
                                          AF.Abs_reciprocal_sqrt)
                    ncA.gpsimd.dma_start(qd.ap()[:, NSL:2 * NSL], dinv)
                else:
                    ncA.scalar.activation(rt[:], dt, AF.Abs_reciprocal_sqrt)
                xs = pool.tile([P, 2 * ccols], bf16, tag="xs")
                ncA.vector.tensor_tensor(
                    out=xs[:].rearrange("p (f c) -> p f c", f=2),
                    in0=xt.rearrange("p (f c) -> p f c", f=2),
                    in1=rt[:, 0:ccols].rearrange("p (a c) -> p a c", a=1)
                        .to_broadcast([P, 2, ccols]),
                    op=ALU.mult)
                xv = xs[:].rearrange("p (f c) -> p f c", f=2)
                for (l0, l1, w, off) in cgroups:
                    if w == 0:
                        continue
                    o = off - c0
                    n = l1 - l0
                    h = w // 2
                    g4 = xv[:, :, o:o + n * w].rearrange(
                        "p f (n w) -> p f n w", w=w)
                    th = pool.tile([P, 2 * n * h], bf16, tag="th")
                    thv = th[:].rearrange("p (f n h) -> p f n h", f=2, h=h)
                    with ncA.allow_low_precision(reason="bf16 agg, e2e-checked"):
                        ncA.vector.tensor_tensor(
                            out=thv, in0=g4[:, :, :, 0:h], in1=g4[:, :, :, h:w],
                            op=ALU.add)
                        ncA.vector.tensor_reduce(
                            out=agg[:].rearrange("p (f n) -> p f n", f=2)[:, :, l0:l1],
                            in_=thv, axis=X, op=ALU.add)

            # z = dinv * agg   (self term is a slot; both features at once)
            z = pp.tile([P, 2 * NSL], bf16)
            ncA.vector.tensor_tensor(
                out=z[:].rearrange("p (f n) -> p f n", f=2),
                in0=agg[:].rearrange("p (f n) -> p f n", f=2),
                in1=dinv.rearrange("p (a n) -> p a n", a=1)
                    .to_broadcast([P, 2, NSL]),
                op=ALU.mult)

            # h_j = relu(a_j z0 + b_j z1 + c_j) = relu(a'_j * (z0 + ratio_j z1) + c_j)
            # acc = sum_j w2_j h_j. All u_j emitted first (DVE runs them back to
            # back), relus pipeline on the scalar engine behind them, then two
            # independent accumulation chains so acc never stalls on a fresh relu.
            z0 = z[:, 0:NSL]
            z1 = z[:, NSL:2 * NSL]
            U = pp.tile([P, 16 * NSL], bf16)
            H = pp.tile([P, 16 * NSL], bf16)
            accA = pp.tile([P, NSL], f32)
            accB = pp.tile([P, NSL], f32)
            for j in range(16):
                ncA.vector.scalar_tensor_tensor(
                    out=U[:, j * NSL:(j + 1) * NSL], in0=z1,
                    scalar=ratio[:, j:j + 1], in1=z0,
                    op0=ALU.mult, op1=ALU.add)
            for j in range(16):
                ncA.scalar.activation(H[:, j * NSL:(j + 1) * NSL],
                                      U[:, j * NSL:(j + 1) * NSL], AF.Relu,
                                      bias=wbt[:, 32 + j:33 + j],
                                      scale=ta[:, j:j + 1])
            for j in range(16):
                dst = accA if j % 2 == 0 else accB
                hj = H[:, j * NSL:(j + 1) * NSL]
                if j < 2:
                    ncA.vector.tensor_scalar(out=dst[:], in0=hj,
                                             scalar1=wbt[:, 48 + j:49 + j],
                                             scalar2=None, op0=ALU.mult)
                else:
                    ncA.vector.scalar_tensor_tensor(
                        out=dst[:], in0=hj, scalar=wbt[:, 48 + j:49 + j],
                        in1=dst[:], op0=ALU.mult, op1=ALU.add)
            ncA.vector.tensor_tensor(out=accA[:], in0=accA[:], in1=accB[:],
                                     op=ALU.add)
            # q = dinv * (h @ W2)
            q = pp.tile([P, NSL], bf16)
            ncA.vector.tensor_tensor(out=q[:], in0=accA[:], in1=dinv,
                                     op=ALU.mult)
            ncA.sync.dma_start(qd.ap()[:, 0:NSL], q[:])
    ncA.compile()

    # ---------------- program B ----------------
    ncB = bacc.Bacc("TRN2", target_bir_lowering=False, debug=False, num_devices=N_CORES)
    sgs = ncB.dram_tensor("sgs", [P, TOT], bf16, kind="ExternalInput")
    dv = ncB.dram_tensor("dv", [P, NSL], bf16, kind="ExternalInput")     # dinv
    bt = ncB.dram_tensor("bt", [P, 1], f32, kind="ExternalInput")        # b2
    out = ncB.dram_tensor("out", [P, NSL], f32, kind="ExternalOutput")

    with tile.TileContext(ncB) as tc:
        with tc.tile_pool(name="slots", bufs=3) as pool, \
             tc.tile_pool(name="persist", bufs=1) as pp:
            dvt = pp.tile([P, NSL], bf16)
            ncB.scalar.dma_start(dvt[:], dv.ap())
            btt = pp.tile([P, 1], f32)
            ncB.scalar.dma_start(btt[:], bt.ap())
            aggS = pp.tile([P, NSL], bf16)
            ncB.gpsimd.memset(aggS[:], 0.0)
            for cgroups in chunksB:
                c0 = cgroups[0][3]
                ccols = sum((e - s) * w for s, e, w, _ in cgroups)
                st = pool.tile([P, ccols], bf16, tag="s")
                ncB.sync.dma_start(st[:], sgs.ap()[:, c0:c0 + ccols])
                for (l0, l1, w, off) in cgroups:
                    if w == 0:
                        continue
                    o = off - c0
                    with ncB.allow_low_precision(reason="bf16 agg, e2e-checked"):
                        ncB.vector.tensor_reduce(
                            out=aggS[:, l0:l1],
                            in_=st[:, o:o + (l1 - l0) * w].rearrange(
                                "p (n w) -> p n w", w=w),
                            axis=X, op=ALU.add)
            o1 = pp.tile([P, NSL], f32)
            ncB.vector.tensor_tensor(out=o1[:], in0=aggS[:], in1=dvt[:],
                                     op=ALU.mult)
            ncB.vector.tensor_scalar(out=o1[:], in0=o1[:],
                                     scalar1=btt[:, 0:1],
                                     scalar2=None, op0=ALU.add)
            ncB.sync.dma_start(out.ap(), o1[:])
    ncB.compile()
    return ncA, ncB, chunksA


def kernel(x, edge_index, W1, b1, W2, b2, n_nodes):
    from concourse.bass_utils import run_bass_kernel_spmd

    N = int(n_nodes)
    x = np.asarray(x, dtype=np.float32)
    ei = np.asarray(edge_index)
    W1 = np.asarray(W1, np.float32); b1 = np.asarray(b1, np.float32)
    W2 = np.asarray(W2, np.float32); b2 = np.asarray(b2, np.float32)
    # self-loops are ordinary slots
    loop = np.arange(N, dtype=np.int64)
    row = np.concatenate([ei[0].astype(np.int64), loop])
    col = np.concatenate([ei[1].astype(np.int64), loop])
    E = row.shape[0]

    # ---- host index layout (structural only) ----
    deg = np.bincount(col, minlength=N)               # includes self-loop
    order = np.argsort(-deg, kind="stable")           # rank -> node
    rank_of = np.empty(N, np.int64)
    rank_of[order] = np.arange(N)

    deg_byrank = np.zeros(NRANKS, np.int64)
    deg_byrank[:N] = deg[order]
    W_l = deg_byrank[np.arange(NSL) * (N_CORES * P)]  # local-slice width
    W_l = W_l + (W_l & 1)                             # even widths (pair adds)
    groups, TOT = _plan_groups(W_l)

    key = (TOT, tuple(g[:3] for g in groups))
    if key not in _cache:
        import os
        if TRACE or os.environ.get("BASS_TRACE"):
            _install_ntff_shim()
        _cache[key] = _build_programs(groups, TOT)
    ncA, ncB, chunksA = _cache[key]

    # per-group lookup tables indexed by local slice
    l2w = np.zeros(NSL, np.int64)
    l2off = np.zeros(NSL, np.int64)   # dgs column of slot (le, k=0)
    for (l0, l1, w, off) in groups:
        l2w[l0:l1] = w
        l2off[l0:l1] = off + (np.arange(l0, l1) - l0) * w
    # chunk tables: for each slice, its chunk's col offset and width
    l2c0 = np.zeros(NSL, np.int64)
    l2cw = np.zeros(NSL, np.int64)
    for cgroups in chunksA:
        c0 = cgroups[0][3]
        ccols = sum((e - s) * w for s, e, w, _ in cgroups)
        for (l0, l1, w, off) in cgroups:
            l2c0[l0:l1] = c0
            l2cw[l0:l1] = ccols

    # ---- per-edge slot placement ----
    re = rank_of[col]
    pe = re & 127
    sl = re >> 7
    ce = sl % N_CORES
    le = sl // N_CORES
    sidx = np.argsort(re, kind="stable")
    re_s = re[sidx]
    runstart = np.empty(E, bool)
    runstart[0] = True
    np.not_equal(re_s[1:], re_s[:-1], out=runstart[1:])
    starts = np.flatnonzero(runstart)
    rid = np.cumsum(runstart) - 1
    slot = np.empty(E, np.int64)
    slot[sidx] = np.arange(E) - starts[rid]
    posd = l2off[le] + slot                    # column in dgs
    # x01s: chunk-feature-major: x0 at 2*c0 + (posd-c0), x1 at +ccols
    posx0 = 2 * l2c0[le] + (posd - l2c0[le])
    posx1 = posx0 + l2cw[le]

    x01s = np.zeros((N_CORES, P, 2 * TOT), BF16)
    dgs = np.ones((N_CORES, P, TOT), np.uint8)
    chunk_meta = []
    boff = 0
    for ci, cgroups in enumerate(_chunk_groups(groups, FRACS_A)):
        c0 = cgroups[0][3]
        ccols = sum((e - s) * w for s, e, w, _ in cgroups)
        extra = NSL if ci == 0 else 0
        chunk_meta.append((c0, ccols, extra, boff))
        boff += ((ccols + extra + 3) & ~3) + 4 * ccols
    core_masks = []
    for c in range(N_CORES):
        m = ce == c
        core_masks.append(m)
        x01s[c][pe[m], posx0[m]] = x[row[m], 0].astype(BF16)
        x01s[c][pe[m], posx1[m]] = x[row[m], 1].astype(BF16)
        dgs[c][pe[m], posd[m]] = deg[row[m]].astype(np.uint8)

    # ---- node tables ----
    pgrid = np.arange(P)[:, None]
    lgrid = np.arange(NSL)[None, :]
    nbs = np.ones((N_CORES, P, NSL), np.uint8)
    nodes_c = []
    valid_c = []
    for c in range(N_CORES):
        ranks = (lgrid * N_CORES + c) * P + pgrid          # [P, NSL]
        valid = ranks < N
        nodes = order[np.minimum(ranks, N - 1)]
        nodes_c.append(nodes); valid_c.append(valid)
        nbs[c] = np.where(valid, deg[nodes], 1).astype(np.uint8)

    wb = np.zeros((P, 65), np.float32)
    wb[:, 0:16] = W1[0]; wb[:, 16:32] = W1[1]
    wb[:, 32:48] = b1
    wb[:, 48:64] = W2[:, 0]
    wb[:, 64] = b2[0]

    abt = np.zeros((N_CORES, P, 5 * TOT + NSL + 16), np.uint8)
    for (c0, ccols, extra, bo) in chunk_meta:
        ao = (ccols + extra + 3) & ~3
        abt[:, :, bo:bo + ccols] = dgs[:, :, c0:c0 + ccols]
        if extra:
            abt[:, :, bo + ccols:bo + ccols + extra] = nbs
        xbytes = np.ascontiguousarray(
            x01s[:, :, 2 * c0:2 * (c0 + ccols)]).view(np.uint8)
        abt[:, :, bo + ao:bo + ao + 4 * ccols] = xbytes
    in_maps_A = [{"abt": abt[c], "wb": wb}
                 for c in range(N_CORES)]
    resA = run_bass_kernel_spmd(ncA, in_maps_A, core_ids=list(range(N_CORES)),
                                trace=TRACE)

    # ---- q table, host re-shard into slots (pure index gather) ----
    q = np.zeros(N, np.float32)
    dinv_full = np.zeros(N, np.float32)
    for c in range(N_CORES):
        v = valid_c[c]
        qdv = resA.results[c]["qd"].astype(np.float32)
        q[nodes_c[c][v]] = qdv[:, 0:NSL][v]
        dinv_full[nodes_c[c][v]] = qdv[:, NSL:2 * NSL][v]
    kernel._dbg = {"q": q, "resA": resA}

    sgs = np.zeros((N_CORES, P, TOT), BF16)
    qrow = q[row].astype(BF16)
    btv = np.full((P, 1), b2[0], np.float32)
    for c in range(N_CORES):
        m = core_masks[c]
        sgs[c][pe[m], posd[m]] = qrow[m]

    in_maps_B = [{"sgs": sgs[c],
                  "dv": dinv_full[nodes_c[c]].astype(BF16) * valid_c[c],
                  "bt": btv}
                 for c in range(N_CORES)]
    resB = run_bass_kernel_spmd(ncB, in_maps_B, core_ids=list(range(N_CORES)),
                                trace=TRACE)

    outv = np.zeros(N, np.float32)
    for c in range(N_CORES):
        v = valid_c[c]
        outv[nodes_c[c][v]] = resB.results[c]["out"][v]
    kernel._dbg.update({"sgs": sgs, "resB": resB,
                        "nodes_c": nodes_c, "valid_c": valid_c,
                        "groups": groups, "TOT": TOT})

    kernel.last_exec_ns = (getattr(resA, "exec_time_ns", None) or 0) + \
                          (getattr(resB, "exec_time_ns", None) or 0)
    return outv[:, None]


# revision 40
# speedup vs baseline: 1.0151x; 1.0151x over previous
"""2-layer GCN (GCNConv -> ReLU -> GCNConv) on 8 TRN2 NeuronCores.

Strategy (sliced-ELLPACK with self-slots, node sharding, bf16, wide DVE ops):
  GCN algebra: out = S relu(S x W1 + b1) W2 + b2 with S = D^-1/2 (A+I) D^-1/2.
  Normalization is separable (norm_e = dinv[row]*dinv[col]) and aggregation
  linear, so each layer is: per-slot scale -> unweighted neighbor-sum ->
  per-node scale -> tiny dense MLP. Self-loops are ordinary slots, so there
  is no separate self-term path.

  Host does pure index layout (no model math): sort nodes by degree, pack
  each node's incoming edges (+1 self slot) into padded slot rows (sliced
  ELLPACK, slices of 128 nodes, DP-chosen width groups). Slot values are the
  *input* features x[row] (bf16) and integer deg[row] (uint8, exact), packed
  into one byte table per DMA chunk (4B-aligned sections, large
  per-partition descriptors); chunk 1 also carries the per-node degs so a
  single rsqrt activation yields both the slot scales and dinv. Device does
  all FP math: rsqrt on the scalar engine (one act-table load, patched so
  Relu shares the abs_reciprocal_sqrt table set), slot scaling (broadcast
  mult) + pair-add + half-width segmented reduction on DVE, then the
  2->16->1 MLP as 16 fused stt ops + relus pipelined on the scalar engine +
  16 fused accumulate ops (DVE-saturated; scalar rides along).

  Two launches: A computes layer 1 + q = dinv * (h @ W2) and exports dinv;
  host re-shards q into the same slot layout (index gather only); B reduces
  q-slots and finishes layer 2 with two DVE ops (no scalar engine at all).
"""
import numpy as np
import ml_dtypes

BF16 = ml_dtypes.bfloat16
P = 128
N_CORES = 8
NSL = 246            # local slices per core (246*8*128 = 251904 >= 250k)
NRANKS = 2048 * P    # padded rank space (262144 >= 250000)
LAM = 60             # DP penalty (columns) per extra width-group
FRACS_A = (0.10, 0.33, 0.63, 1.01)   # graded DMA chunks, program A
FRACS_B = (0.20, 0.55, 1.01)   # program B

TRACE = False
_cache = {}


def _install_ntff_shim():
    import contextlib, ctypes, sys, types
    if "antenv.axon_hooks" in sys.modules:
        return
    try:
        lib = ctypes.CDLL("/opt/axon/libaxon_pjrt.so")
        if not hasattr(lib, "axon_start_nrt_profile"):
            return
        lib.axon_start_nrt_profile.argtypes = [ctypes.POINTER(ctypes.c_int64), ctypes.c_size_t]
        lib.axon_start_nrt_profile.restype = ctypes.c_int64
        lib.axon_stop_nrt_profile.argtypes = [ctypes.c_char_p]
        lib.axon_stop_nrt_profile.restype = ctypes.c_int64
    except OSError:
        return

    @contextlib.contextmanager
    def _hook(output_dir, device_ids):
        import jax
        jax.devices()
        if device_ids:
            ids = (ctypes.c_int64 * len(device_ids))(*device_ids)
            rc = lib.axon_start_nrt_profile(ids, len(device_ids))
        else:
            rc = lib.axon_start_nrt_profile(None, 0)
        if rc != 0:
            raise RuntimeError(f"axon_start_nrt_profile rc={rc}")
        try:
            yield
        finally:
            lib.axon_stop_nrt_profile(str(output_dir).encode())

    mod = types.ModuleType("antenv.axon_hooks")
    mod.get_axon_ntff_profile_hook = lambda: _hook
    mod.set_axon_ntff_profile_hook = lambda h: None
    sys.modules["antenv.axon_hooks"] = mod


def _plan_groups(W_l):
    """DP partition of local slices into contiguous width-groups.
    Boundaries restricted to even slice indices so every group block has an
    even column count (keeps bf16 sub-blocks 4B-aligned for DVE 2x mode)."""
    nsl = len(W_l)
    INF = 1 << 60
    best = np.full(nsl + 1, INF, np.int64)
    best[0] = 0
    ch = np.zeros(nsl + 1, np.int64)
    for e in range(2, nsl + 1, 2):
        s = np.arange(0, e, 2)
        c = best[s] + (e - s) * W_l[s] + LAM
        i = int(np.argmin(c))
        best[e] = c[i]
        ch[e] = 2 * i
    bnds = []
    e = nsl
    while e > 0:
        s = int(ch[e])
        bnds.append((s, e))
        e = s
    bnds.reverse()
    groups = []
    off = 0
    for s, e in bnds:
        w = int(W_l[s])
        groups.append((s, e, w, off))   # (l0, l1, width, column offset)
        off += (e - s) * w
    return groups, off                  # TOT = off


def _chunk_groups(groups, fracs):
    """Split groups into DMA chunks with graded sizes (small first chunk so
    compute starts as early as possible while later transfers overlap)."""
    tot = sum((e - s) * w for s, e, w, _ in groups)
    chunks = []
    cur = []
    acc = 0
    for g in groups:
        s, e, w, _ = g
        cur.append(g)
        acc += (e - s) * w
        if len(chunks) < len(fracs) - 1 and acc >= fracs[len(chunks)] * tot:
            chunks.append(cur)
            cur = []
    if cur:
        chunks.append(cur)
    return [c for c in chunks if c]


def _patch_act_tables():
    """Route both Relu and Abs_reciprocal_sqrt to the single act-func set
    that contains them both, so only one ACT_TABLE_LOAD (1.28us) is emitted.
    Entries before the target set are hidden (emptied) so positional
    act_func_set_ids stay aligned with act_info.json."""
    import concourse.bacc as bacc
    from concourse.hw_specs import get_activation_tables as _gat
    if getattr(bacc, "_act_tables_patched", False):
        return
    def patched(arch):
        tabs = _gat(arch)
        out = {}
        seen = False
        for k, v in tabs.items():
            if k == "abs_reciprocal_sqrt_and_small":
                out[k] = v
                seen = True
            elif not seen:
                out[k] = set()
            else:
                out[k] = v
        return out
    bacc.get_activation_tables = patched
    bacc._act_tables_patched = True


def _build_programs(groups, TOT):
    import concourse.bass as bass
    import concourse.bacc as bacc
    import concourse.tile as tile
    import concourse.mybir as mybir

    _patch_act_tables()

    f32 = mybir.dt.float32
    bf16 = mybir.dt.bfloat16
    AF = mybir.ActivationFunctionType
    ALU = mybir.AluOpType
    X = mybir.AxisListType.X
    chunksA = _chunk_groups(groups, FRACS_A)
    chunksB = _chunk_groups(groups, FRACS_B)

    # ---------------- program A ----------------
    ncA = bacc.Bacc("TRN2", target_bir_lowering=False, debug=False, num_devices=N_CORES)
    u8 = mybir.dt.uint8
    # packed per-chunk byte table: [deg u8 (ccols) | x0x1 bf16 (4*ccols B)]
    abt = ncA.dram_tensor("abt", [P, 5 * TOT + NSL + 16], u8, kind="ExternalInput")
    wb = ncA.dram_tensor("wb", [P, 65], f32, kind="ExternalInput")
    qd = ncA.dram_tensor("qd", [P, 2 * NSL], bf16, kind="ExternalOutput")  # q | dinv

    with tile.TileContext(ncA) as tc:
        with tc.tile_pool(name="slots", bufs=4) as pool, \
             tc.tile_pool(name="persist", bufs=1) as pp:
            # one packed transfer per chunk (large per-partition descriptors);
            # x section padded to a 4-byte boundary for DVE fast modes
            cts = []
            boff = 0
            for ci, cgroups in enumerate(chunksA):
                ccols = sum((e - s) * w for s, e, w, _ in cgroups)
                extra = NSL if ci == 0 else 0    # chunk 1 carries node degs
                aoff = (ccols + extra + 3) & ~3
                stride = aoff + 4 * ccols
                ct = pool.tile([P, stride], u8, tag="ab")
                if ci == 0:
                    # deg+nodedeg section first so rt1/dinv start early,
                    # x section follows on the same queue
                    ncA.sync.dma_start(ct[:, 0:aoff],
                                       abt.ap()[:, boff:boff + aoff])
                    ncA.sync.dma_start(ct[:, aoff:stride],
                                       abt.ap()[:, boff + aoff:boff + stride])
                else:
                    ncA.sync.dma_start(ct[:], abt.ap()[:, boff:boff + stride])
                cts.append(ct)
                boff += stride
            wbt = pp.tile([P, 65], f32)
            ncA.scalar.dma_start(wbt[:], wb.ap())
            agg = pp.tile([P, 2 * NSL], bf16)
            ncA.gpsimd.memset(agg[:], 0.0)

            # per-j MLP constants: relu scale ta_j = a_j,
            # ratio_j = b_j / a_j (u_j = z0 + ratio_j z1 folded via relu scale)
            ta = pp.tile([P, 16], f32)
            ncA.vector.tensor_scalar(out=ta[:], in0=wbt[:, 0:16], scalar1=1e-30,
                                     scalar2=None, op0=ALU.add)
            ratio = pp.tile([P, 16], f32)
            ncA.vector.reciprocal(ratio[:], ta[:])
            ncA.vector.tensor_tensor(out=ratio[:], in0=wbt[:, 16:32], in1=ratio[:],
                                     op=ALU.mult)

            # slot chunks: rsqrt(deg[row]) on scalar, scale + segmented-sum
            # on DVE. x01s chunk layout: [x0 of chunk | x1 of chunk]
            dinv = None
            for ci, cgroups in enumerate(chunksA):
                c0 = cgroups[0][3]
                ccols = sum((e - s) * w for s, e, w, _ in cgroups)
                ct = cts[ci]
                extra = NSL if ci == 0 else 0
                aoff = (ccols + extra + 3) & ~3
                dt = ct[:, 0:ccols + extra]
                xt = ct[:, aoff:aoff + 4 * ccols].bitcast(bf16)
                rt = pool.tile([P, ccols + extra], bf16, tag="r")
                if ci == 0:
                    # slot scales first (gates scale1); node dinv separately
                    # (needed only at z time, exported early off-queue)
                    ncA.scalar.activation(rt[:, 0:ccols], ct[:, 0:ccols],
                                          AF.Abs_reciprocal_sqrt)
                    dinv = rt[:, ccols:ccols + NSL]
                    ncA.scalar.activation(dinv, ct[:, ccols:ccols + NSL],
                                          AF.Abs_reciprocal_sqrt)
                    ncA.gpsimd.dma_start(qd.ap()[:, NSL:2 * NSL], dinv)
                else:
                    ncA.scalar.activation(rt[:], dt, AF.Abs_reciprocal_sqrt)
                xs = pool.tile([P, 2 * ccols], bf16, tag="xs")
                ncA.vector.tensor_tensor(
                    out=xs[:].rearrange("p (f c) -> p f c", f=2),
                    in0=xt.rearrange("p (f c) -> p f c", f=2),
                    in1=rt[:, 0:ccols].rearrange("p (a c) -> p a c", a=1)
                        .to_broadcast([P, 2, ccols]),
                    op=ALU.mult)
                xv = xs[:].rearrange("p (f c) -> p f c", f=2)
                for (l0, l1, w, off) in cgroups:
                    if w == 0:
                        continue
                    o = off - c0
                    n = l1 - l0
                    h = w // 2
                    g4 = xv[:, :, o:o + n * w].rearrange(
                        "p f (n w) -> p f n w", w=w)
                    th = pool.tile([P, 2 * n * h], bf16, tag="th")
                    thv = th[:].rearrange("p (f n h) -> p f n h", f=2, h=h)
                    with ncA.allow_low_precision(reason="bf16 agg, e2e-checked"):
                        ncA.vector.tensor_tensor(
                            out=thv, in0=g4[:, :, :, 0:h], in1=g4[:, :, :, h:w],
                            op=ALU.add)
                        ncA.vector.tensor_reduce(
                            out=agg[:].rearrange("p (f n) -> p f n", f=2)[:, :, l0:l1],
                            in_=thv, axis=X, op=ALU.add)

            # z = dinv * agg   (self term is a slot; both features at once)
            z = pp.tile([P, 2 * NSL], bf16)
            ncA.vector.tensor_tensor(
                out=z[:].rearrange("p (f n) -> p f n", f=2),
                in0=agg[:].rearrange("p (f n) -> p f n", f=2),
                in1=dinv.rearrange("p (a n) -> p a n", a=1)
                    .to_broadcast([P, 2, NSL]),
                op=ALU.mult)

            # h_j = relu(a_j z0 + b_j z1 + c_j) = relu(a'_j * (z0 + ratio_j z1) + c_j)
            # acc = sum_j w2_j h_j. All u_j emitted first (DVE runs them back to
            # back), relus pipeline on the scalar engine behind them, then two
            # independent accumulation chains so acc never stalls on a fresh relu.
            z0 = z[:, 0:NSL]
            z1 = z[:, NSL:2 * NSL]
            U = pp.tile([P, 16 * NSL], bf16)
            H = pp.tile([P, 16 * NSL], bf16)
            accA = pp.tile([P, NSL], f32)
            accB = pp.tile([P, NSL], f32)
            for j in range(16):
                ncA.vector.scalar_tensor_tensor(
                    out=U[:, j * NSL:(j + 1) * NSL], in0=z1,
                    scalar=ratio[:, j:j + 1], in1=z0,
                    op0=ALU.mult, op1=ALU.add)
            for j in range(16):
                ncA.scalar.activation(H[:, j * NSL:(j + 1) * NSL],
                                      U[:, j * NSL:(j + 1) * NSL], AF.Relu,
                                      bias=wbt[:, 32 + j:33 + j],
                                      scale=ta[:, j:j + 1])
            for j in range(16):
                dst = accA if j % 2 == 0 else accB
                hj = H[:, j * NSL:(j + 1) * NSL]
                if j < 2:
                    ncA.vector.tensor_scalar(out=dst[:], in0=hj,
                                             scalar1=wbt[:, 48 + j:49 + j],
                                             scalar2=None, op0=ALU.mult)
                else:
                    ncA.vector.scalar_tensor_tensor(
                        out=dst[:], in0=hj, scalar=wbt[:, 48 + j:49 + j],
                        in1=dst[:], op0=ALU.mult, op1=ALU.add)
            ncA.vector.tensor_tensor(out=accA[:], in0=accA[:], in1=accB[:],
                                     op=ALU.add)
            # q = dinv * (h @ W2)
            q = pp.tile([P, NSL], bf16)
            ncA.vector.tensor_tensor(out=q[:], in0=accA[:], in1=dinv,
                                     op=ALU.mult)
            ncA.sync.dma_start(qd.ap()[:, 0:NSL], q[:])
    ncA.compile()

    # ---------------- program B ----------------
    ncB = bacc.Bacc("TRN2", target_bir_lowering=False, debug=False, num_devices=N_CORES)
    sgs = ncB.dram_tensor("sgs", [P, TOT], bf16, kind="ExternalInput")
    dv = ncB.dram_tensor("dv", [P, NSL], bf16, kind="ExternalInput")     # dinv
    bt = ncB.dram_tensor("bt", [P, 1], f32, kind="ExternalInput")        # b2
    out = ncB.dram_tensor("out", [P, NSL], f32, kind="ExternalOutput")

    with tile.TileContext(ncB) as tc:
        with tc.tile_pool(name="slots", bufs=3) as pool, \
             tc.tile_pool(name="persist", bufs=1) as pp:
            dvt = pp.tile([P, NSL], bf16)
            ncB.scalar.dma_start(dvt[:], dv.ap())
            btt = pp.tile([P, 1], f32)
            ncB.scalar.dma_start(btt[:], bt.ap())
            aggS = pp.tile([P, NSL], bf16)
            ncB.gpsimd.memset(aggS[:], 0.0)
            for cgroups in chunksB:
                c0 = cgroups[0][3]
                ccols = sum((e - s) * w for s, e, w, _ in cgroups)
                st = pool.tile([P, ccols], bf16, tag="s")
                ncB.sync.dma_start(st[:], sgs.ap()[:, c0:c0 + ccols])
                for (l0, l1, w, off) in cgroups:
                    if w == 0:
                        continue
                    o = off - c0
                    with ncB.allow_low_precision(reason="bf16 agg, e2e-checked"):
                        ncB.vector.tensor_reduce(
                            out=aggS[:, l0:l1],
                            in_=st[:, o:o + (l1 - l0) * w].rearrange(
                                "p (n w) -> p n w", w=w),
                            axis=X, op=ALU.add)
            o1 = pp.tile([P, NSL], f32)
            ncB.vector.tensor_tensor(out=o1[:], in0=aggS[:], in1=dvt[:],
                                     op=ALU.mult)
            ncB.vector.tensor_scalar(out=o1[:], in0=o1[:],
                                     scalar1=btt[:, 0:1],
                                     scalar2=None, op0=ALU.add)
            ncB.sync.dma_start(out.ap(), o1[:])
    ncB.compile()
    return ncA, ncB, chunksA


def kernel(x, edge_index, W1, b1, W2, b2, n_nodes):
    from concourse.bass_utils import run_bass_kernel_spmd

    N = int(n_nodes)
    x = np.asarray(x, dtype=np.float32)
    ei = np.asarray(edge_index)
    W1 = np.asarray(W1, np.float32); b1 = np.asarray(b1, np.float32)
    W2 = np.asarray(W2, np.float32); b2 = np.asarray(b2, np.float32)
    # self-loops are ordinary slots
    loop = np.arange(N, dtype=np.int64)
    row = np.concatenate([ei[0].astype(np.int64), loop])
    col = np.concatenate([ei[1].astype(np.int64), loop])
    E = row.shape[0]

    # ---- host index layout (structural only) ----
    deg = np.bincount(col, minlength=N)               # includes self-loop
    order = np.argsort(-deg, kind="stable")           # rank -> node
    rank_of = np.empty(N, np.int64)
    rank_of[order] = np.arange(N)

    deg_byrank = np.zeros(NRANKS, np.int64)
    deg_byrank[:N] = deg[order]
    W_l = deg_byrank[np.arange(NSL) * (N_CORES * P)]  # local-slice width
    W_l = W_l + (W_l & 1)                             # even widths (pair adds)
    groups, TOT = _plan_groups(W_l)

    key = (TOT, tuple(g[:3] for g in groups))
    if key not in _cache:
        import os
        if TRACE or os.environ.get("BASS_TRACE"):
            _install_ntff_shim()
        _cache[key] = _build_programs(groups, TOT)
    ncA, ncB, chunksA = _cache[key]

    # per-group lookup tables indexed by local slice
    l2w = np.zeros(NSL, np.int64)
    l2off = np.zeros(NSL, np.int64)   # dgs column of slot (le, k=0)
    for (l0, l1, w, off) in groups:
        l2w[l0:l1] = w
        l2off[l0:l1] = off + (np.arange(l0, l1) - l0) * w
    # chunk tables: for each slice, its chunk's col offset and width
    l2c0 = np.zeros(NSL, np.int64)
    l2cw = np.zeros(NSL, np.int64)
    for cgroups in chunksA:
        c0 = cgroups[0][3]
        ccols = sum((e - s) * w for s, e, w, _ in cgroups)
        for (l0, l1, w, off) in cgroups:
            l2c0[l0:l1] = c0
            l2cw[l0:l1] = ccols

    # ---- per-edge slot placement ----
    re = rank_of[col]
    pe = re & 127
    sl = re >> 7
    ce = sl % N_CORES
    le = sl // N_CORES
    sidx = np.argsort(re, kind="stable")
    re_s = re[sidx]
    runstart = np.empty(E, bool)
    runstart[0] = True
    np.not_equal(re_s[1:], re_s[:-1], out=runstart[1:])
    starts = np.flatnonzero(runstart)
    rid = np.cumsum(runstart) - 1
    slot = np.empty(E, np.int64)
    slot[sidx] = np.arange(E) - starts[rid]
    posd = l2off[le] + slot                    # column in dgs
    # x01s: chunk-feature-major: x0 at 2*c0 + (posd-c0), x1 at +ccols
    posx0 = 2 * l2c0[le] + (posd - l2c0[le])
    posx1 = posx0 + l2cw[le]

    x01s = np.zeros((N_CORES, P, 2 * TOT), BF16)
    dgs = np.ones((N_CORES, P, TOT), np.uint8)
    chunk_meta = []
    boff = 0
    for ci, cgroups in enumerate(_chunk_groups(groups, FRACS_A)):
        c0 = cgroups[0][3]
        ccols = sum((e - s) * w for s, e, w, _ in cgroups)
        extra = NSL if ci == 0 else 0
        chunk_meta.append((c0, ccols, extra, boff))
        boff += ((ccols + extra + 3) & ~3) + 4 * ccols
    core_masks = []
    for c in range(N_CORES):
        m = ce == c
        core_masks.append(m)
        x01s[c][pe[m], posx0[m]] = x[row[m], 0].astype(BF16)
        x01s[c][pe[m], posx1[m]] = x[row[m], 1].astype(BF16)
        dgs[c][pe[m], posd[m]] = deg[row[m]].astype(np.uint8)

    # ---- node tables ----
    pgrid = np.arange(P)[:, None]
    lgrid = np.arange(NSL)[None, :]
    nbs = np.ones((N_CORES, P, NSL), np.uint8)
    nodes_c = []
    valid_c = []
    for c in range(N_CORES):
        ranks = (lgrid * N_CORES + c) * P + pgrid          # [P, NSL]
        valid = ranks < N
        nodes = order[np.minimum(ranks, N - 1)]
        nodes_c.append(nodes); valid_c.append(valid)
        nbs[c] = np.where(valid, deg[nodes], 1).astype(np.uint8)

    wb = np.zeros((P, 65), np.float32)
    wb[:, 0:16] = W1[0]; wb[:, 16:32] = W1[1]
    wb[:, 32:48] = b1
    wb[:, 48:64] = W2[:, 0]
    wb[:, 64] = b2[0]

    abt = np.zeros((N_CORES, P, 5 * TOT + NSL + 16), np.uint8)
    for (c0, ccols, extra, bo) in chunk_meta:
        ao = (ccols + extra + 3) & ~3
        abt[:, :, bo:bo + ccols] = dgs[:, :, c0:c0 + ccols]
        if extra:
            abt[:, :, bo + ccols:bo + ccols + extra] = nbs
        xbytes = np.ascontiguousarray(
            x01s[:, :, 2 * c0:2 * (c0 + ccols)]).view(np.uint8)
        abt[:, :, bo + ao:bo + ao + 4 * ccols] = xbytes
    in_maps_A = [{"abt": abt[c], "wb": wb}
                 for c in range(N_CORES)]
    resA = run_bass_kernel_spmd(ncA, in_maps_A, core_ids=list(range(N_CORES)),
                                trace=TRACE)

    # ---- q table, host re-shard into slots (pure index gather) ----
    q = np.zeros(N, np.float32)
    dinv_full = np.zeros(N, np.float32)
    for c in range(N_CORES):
        v = valid_c[c]
        qdv = resA.results[c]["qd"].astype(np.float32)
        q[nodes_c[c][v]] = qdv[:, 0:NSL][v]
        dinv_full[nodes_c[c][v]] = qdv[:, NSL:2 * NSL][v]
    kernel._dbg = {"q": q, "resA": resA}

    sgs = np.zeros((N_CORES, P, TOT), BF16)
    qrow = q[row].astype(BF16)
    btv = np.full((P, 1), b2[0], np.float32)
    for c in range(N_CORES):
        m = core_masks[c]
        sgs[c][pe[m], posd[m]] = qrow[m]

    in_maps_B = [{"sgs": sgs[c],
                  "dv": dinv_full[nodes_c[c]].astype(BF16) * valid_c[c],
                  "bt": btv}
                 for c in range(N_CORES)]
    resB = run_bass_kernel_spmd(ncB, in_maps_B, core_ids=list(range(N_CORES)),
                                trace=TRACE)

    outv = np.zeros(N, np.float32)
    for c in range(N_CORES):
        v = valid_c[c]
        outv[nodes_c[c][v]] = resB.results[c]["out"][v]
    kernel._dbg.update({"sgs": sgs, "resB": resB,
                        "nodes_c": nodes_c, "valid_c": valid_c,
                        "groups": groups, "TOT": TOT})

    kernel.last_exec_ns = (getattr(resA, "exec_time_ns", None) or 0) + \
                          (getattr(resB, "exec_time_ns", None) or 0)
    return outv[:, None]
